# revision 1
# baseline (speedup 1.0000x reference)
"""Multi-head attention (B=4, L=2048, C=1024, H=16, D=64) on 8 TRN2 NeuronCores.

Sharding: core c handles batch b = c//2 and head-group hg = c%2 (8 heads).
Megatron-style: w_qkv column-sharded, w_proj row-sharded; the proj all-reduce
(2 cores per batch) happens on the host during unshard.

Per-core dataflow (all matmuls fp32r = full PE rate, ~1e-4 rel err):
  - host passes x[b] TRANSPOSED (xT [c, l]) plus pre-transposed/permuted
    weight slices, so the device needs zero transposes:
      q/k proj:  qT[f, l]  = (wqkT chunk).T @ xT     (f on partitions)
      v proj:    V[l, fv]  = (xT chunk).T @ wvT      (natural, for attn@V lhsT)
      scores:    S^T[k, q] = kT.T @ qT   (contraction d=64; two heads packed
                 per 128 partitions, row-tiled via tile_position)
      softmax:   exp on ScalarE straight out of PSUM (scale=1/8 fused);
                 no max-subtraction needed (|scores| <~ 6)
      attn@V:    O^T[d, q] = V'.T @ P^T accumulated over 16 k-chunks, with a
                 ones column in V' producing the softmax denominator in row 64
      norm:      reciprocal + gpsimd partition_broadcast + fused mul (DVE)
      proj:      out[l, co] = (O^T chunk).T @ wpT, partial over 512 dims
  - RoPE: w_qkv rows for q/k are host-permuted so that within each
    32-partition quadrant, even-d lanes sit at rows 0:16 and odd-d at 16:32.
    Then rope = qps*cos4 + quadrant_swap(qps)*sin4s, where the swap is a
    single DVE stream_shuffle and cos4/sin4s are host-built broadcast tables
    (sign folded into sin4s).
"""

import sys

sys.path.insert(0, "/opt/trn_rl_repo")

import numpy as np

B, L, C, H, D = 4, 2048, 1024, 16, 64
NCORES = 8
QT = 512          # q-tile; score mm N=512 = one full PSUM bank (HW requires one tile_position mm per bank)
GRP = 2           # score chunk-banks per exp group
PSSC_BUFS = 2
PT_BUFS = 4
QK_BUFS = 3
OTP_BUFS = 2
NORM = "dve"      # softmax-denominator broadcast impl: dve | gpsimd | none
PSQP_BUFS = 2
ABLATE = set()    # debug: subset of {"exp", "rope", "av", "interleave", "phasec"}
_built = {}


def _build(nc_mod):
    """Build the per-core Bass program (identical on all cores)."""
    import concourse.mybir as mybir
    import concourse.tile as tile
    from concourse import bacc
    from concourse.alu_op_type import AluOpType

    F32 = mybir.dt.float32
    F32R = mybir.dt.float32r
    EXP = mybir.ActivationFunctionType.Exp
    MULT = AluOpType.mult
    ADD = AluOpType.add
    BYPASS = AluOpType.bypass

    NKC = C // 128          # 8 contraction chunks for qkv proj
    NLT = L // 128          # 16 l-tiles (V rows, proj rows, k-chunks)
    NQT = L // QT           # q-tiles per pair
    NPAIR = 4               # head pairs per core
    FV = 512                # v features per core
    VW = 65                 # V columns incl. ones

    nc = bacc.Bacc(None, target_bir_lowering=False)

    xT_d = nc.dram_tensor("xT", [128, NKC, L], F32R, kind="ExternalInput")
    wqkT_d = nc.dram_tensor("wqkT", [8, 128, NKC, 128], F32R, kind="ExternalInput")
    wvT_d = nc.dram_tensor("wvT", [NKC, 128, FV], F32R, kind="ExternalInput")
    wpT_d = nc.dram_tensor("wpT", [128, NPAIR, C], F32R, kind="ExternalInput")
    cos4_d = nc.dram_tensor("cos4", [128, L], F32, kind="ExternalInput")
    sin4_d = nc.dram_tensor("sin4", [128, L], F32, kind="ExternalInput")
    outp_d = nc.dram_tensor("outp", [NLT, 128, C], F32, kind="ExternalOutput")

    SWAP_MASK = list(range(16, 32)) + list(range(16))

    with tile.TileContext(nc) as tc:
        import contextlib

        with contextlib.ExitStack() as outer:
            persist = outer.enter_context(tc.tile_pool(name="persist", bufs=1))
            qk_pool = outer.enter_context(tc.tile_pool(name="qkt", bufs=QK_BUFS))
            ot_pool = outer.enter_context(tc.tile_pool(name="otp", bufs=OTP_BUFS))
            dram = outer.enter_context(tc.tile_pool(name="dram", bufs=1, space="DRAM"))
            OT_dram = dram.tile([128, NPAIR, L], F32R)

            with contextlib.ExitStack() as mid:
                xpool = mid.enter_context(tc.tile_pool(name="xT", bufs=1))
                wvpool = mid.enter_context(tc.tile_pool(name="wV", bufs=1))
                wpool = mid.enter_context(tc.tile_pool(name="wA", bufs=2))
                cpool = mid.enter_context(tc.tile_pool(name="csn", bufs=1))
                tpool = mid.enter_context(tc.tile_pool(name="tmp", bufs=2))
                pt_pool = mid.enter_context(tc.tile_pool(name="pt", bufs=PT_BUFS))
                ps_sc = mid.enter_context(tc.tile_pool(name="ps_sc", bufs=PSSC_BUFS, space="PSUM"))
                ps_av = mid.enter_context(tc.tile_pool(name="ps_av", bufs=1, space="PSUM"))
                ps_qp = mid.enter_context(tc.tile_pool(name="ps_qp", bufs=PSQP_BUFS, space="PSUM"))

                # ---- persistent tensors ----
                V_t = persist.tile([128, NLT, 8, VW], F32R, tag="V")

                # ---- input DMAs (wvT first: the A1 matmuls need all of it) ----
                wvT_t = wvpool.tile([128, NKC, FV], F32R, tag="wv")
                xT_t = xpool.tile([128, NKC, L], F32R)
                nc.sync.dma_start(xT_t[:, :, 0:128], xT_d[:, :, 0:128])
                for kc in range(NKC):
                    nc.sync.dma_start(wvT_t[:, kc, :], wvT_d[kc])
                for lt in range(1, NLT):
                    sl = slice(lt * 128, (lt + 1) * 128)
                    nc.sync.dma_start(xT_t[:, :, sl], xT_d[:, :, sl])
                cos4_t = cpool.tile([128, L], F32)
                sin4_t = cpool.tile([128, L], F32)
                ones_t = cpool.tile([128, NLT, 8], F32)
                nc.vector.memset(ones_t[:], 1.0)
                nc.vector.tensor_copy(V_t[:, :, :, 64:65], ones_t[:, :, :, None])
                norm_scr = []
                for i in range(4):
                    nsc = cpool.tile([64, QT], F32, tag=f"nsc{i}")
                    nc.vector.memset(nsc[:], 1.0)
                    norm_scr.append(nsc)

                # ---- phase A1: V = x @ Wv (natural layout) ----
                for lt in range(NLT):
                    vps = ps_qp.tile([128, 512], F32, tag="qps")
                    for kc in range(NKC):
                        nc.tensor.matmul(
                            vps[:],
                            xT_t[:, kc, lt * 128:(lt + 1) * 128],
                            wvT_t[:, kc, :],
                            start=(kc == 0),
                            stop=(kc == NKC - 1),
                        )
                    nc.vector.tensor_copy(V_t[:, lt, :, 0:64], vps[:])

                nc.sync.dma_start(cos4_t[:], cos4_d[:])
                nc.sync.dma_start(sin4_t[:], sin4_d[:])

                # ---- phase A2 helper: qT/kT for one f-tile (one pair, q or k) ----
                qkT = {}

                def emit_qk(ft):
                    wqk = wpool.tile([128, NKC, 128], F32R, tag="wqk")
                    nc.sync.dma_start(wqk[:], wqkT_d[ft])
                    dst = qk_pool.tile([128, L], F32R, tag="qkt")
                    qkT[ft] = dst
                    for lq in range(L // 512):
                        qps = ps_qp.tile([128, 512], F32, tag="qps")
                        sl = slice(lq * 512, (lq + 1) * 512)
                        for kc in range(NKC):
                            nc.tensor.matmul(
                                qps[:],
                                wqk[:, kc, :],
                                xT_t[:, kc, sl],
                                start=(kc == 0),
                                stop=(kc == NKC - 1),
                            )
                        if "rope" in ABLATE:
                            nc.vector.tensor_copy(dst[:, sl], qps[:])
                        else:
                            # RoPE: dst = qps*cos4 + swap(qps)*sin4s
                            shuf = tpool.tile([128, 512], F32, tag="shuf")
                            nc.vector.stream_shuffle(shuf[:], qps[:], SWAP_MASK)
                            nc.vector.tensor_tensor(dst[:, sl], qps[:], cos4_t[:, sl], op=MULT)
                            nc.vector.tensor_tensor(shuf[:], shuf[:], sin4_t[:, sl], op=MULT)
                            nc.vector.tensor_tensor(dst[:, sl], dst[:, sl], shuf[:], op=ADD)

                emit_qk(0)
                emit_qk(4)

                if "only_a" in ABLATE:
                    nc.sync.dma_start(outp_d[0, :, 0:512].bitcast(F32R), qkT[0][:, 0:512])
                    nc.sync.dma_start(outp_d[1, :, 0:512].bitcast(F32R), qkT[4][:, 0:512])
                    for ft in [1, 5, 2, 6, 3, 7]:
                        emit_qk(ft)
                        nc.sync.dma_start(
                            outp_d[ft % NLT, :, 0:512].bitcast(F32R), qkT[ft][:, 0:512]
                        )

                # ---- phase B: attention per (pair, q-tile), A2 interleaved ----
                NGRP = 2 * NLT // GRP  # exp groups per pair-qtile
                for pr in range(NPAIR if "only_a" not in ABLATE else 0):
                    qT_t, kT_t = qkT[pr], qkT[4 + pr]
                    OT_t = ot_pool.tile([128, L], F32R, tag="otp")
                    for qt in range(NQT):
                        qsl = slice(qt * QT, (qt + 1) * QT)
                        avA = ps_av.tile([128, QT], F32, tag="avA")
                        avB = ps_av.tile([128, QT], F32, tag="avB")
                        av = [avA, avB]
                        for g0 in range(0, 2 * NLT, GRP):
                            glen = min(GRP, 2 * NLT - g0)
                            sc = ps_sc.tile([128, GRP, QT], F32, tag="sc")
                            pt = pt_pool.tile([128, GRP, QT], F32R, tag="pt")
                            for j in range(glen):
                                s = g0 + j
                                kt, hd = s // 2, s % 2
                                if "noscore" in ABLATE:
                                    nc.tensor.matmul(
                                        sc[:, j, :],
                                        kT_t[0:128, kt * 128:(kt + 1) * 128],
                                        qT_t[0:128, qsl],
                                        start=True,
                                        stop=True,
                                    )
                                    continue
                                nc.tensor.matmul(
                                    sc[:, j, :],
                                    kT_t[hd * 64:(hd + 1) * 64, kt * 128:(kt + 1) * 128],
                                    qT_t[hd * 64:(hd + 1) * 64, qsl],
                                    start=True,
                                    stop=True,
                                    tile_position=(hd * 64, 0),
                                )
                            if "exp" in ABLATE:
                                nc.vector.tensor_copy(pt[:, 0:glen, :], sc[:, 0:glen, :])
                            else:
                                nc.scalar.activation(pt[:, 0:glen, :], sc[:, 0:glen, :], EXP, scale=float(D) ** -0.5)
                            for j in range(glen):
                                s = g0 + j
                                kt, hd = s // 2, s % 2
                                if "av" in ABLATE:
                                    continue
                                nc.tensor.matmul(
                                    av[hd][0:VW, :],
                                    V_t[:, kt, pr * 2 + hd, :],
                                    pt[:, j, :],
                                    start=(kt == 0),
                                    stop=(kt == NLT - 1),
                                )
                        # normalize and write O^T
                        for hd in range(2):
                            if "av" in ABLATE:
                                nc.vector.tensor_copy(
                                    OT_t[hd * 64:(hd + 1) * 64, qsl], pt[0:64, 0, :]
                                )
                                continue
                            if NORM == "none":
                                nc.vector.tensor_copy(
                                    OT_t[hd * 64:(hd + 1) * 64, qsl], av[hd][0:64, :]
                                )
                                continue
                            if NORM == "gpsimd":
                                rd = tpool.tile([1, QT], F32, tag="rd")
                                nc.vector.reciprocal(rd[:], av[hd][64:65, :])
                                db = tpool.tile([64, QT], F32, tag="db")
                                nc.gpsimd.partition_broadcast(db[:], rd[:])
                            else:  # dve
                                rd = norm_scr[2 * hd]
                                nc.vector.reciprocal(rd[0:1, :], av[hd][64:65, :])
                                nc.vector.reciprocal(rd[32:33, :], av[hd][64:65, :])
                                db = norm_scr[2 * hd + 1]
                                nc.vector.stream_shuffle(db[:], rd[:], [0] * 32)
                            nc.vector.scalar_tensor_tensor(
                                OT_t[hd * 64:(hd + 1) * 64, qsl],
                                av[hd][0:64, :],
                                1.0,
                                db[:],
                                op0=MULT,
                                op1=MULT,
                            )
                        nc.sync.dma_start(OT_dram[:, pr, qsl], OT_t[:, qsl])
                        if pr < NPAIR - 1 and "interleave" not in ABLATE:
                            if qt == min(1, NQT - 1):
                                emit_qk(pr + 1)
                            if qt == min(3, NQT - 1):
                                emit_qk(5 + pr)
                    if pr < NPAIR - 1 and "interleave" in ABLATE:
                        emit_qk(pr + 1)
                        emit_qk(5 + pr)



            # ---- phase C: partial out-proj ----
            with contextlib.ExitStack() as cstack:
                ob_pool = cstack.enter_context(tc.tile_pool(name="ob", bufs=4))
                ps_c = cstack.enter_context(tc.tile_pool(name="ps_c", bufs=4, space="PSUM"))
                wp_pool = cstack.enter_context(tc.tile_pool(name="wp", bufs=1))
                otc_pool = cstack.enter_context(tc.tile_pool(name="otc", bufs=6))
                wpT_t = wp_pool.tile([128, NPAIR, C], F32R)
                for kd in range(NPAIR):
                    nc.sync.dma_start(wpT_t[:, kd, :], wpT_d[:, kd, :])
                for lt in range(NLT if "phasec" not in ABLATE else 0):
                    lsl = slice(lt * 128, (lt + 1) * 128)
                    ot_c = otc_pool.tile([128, NPAIR, 128], F32R, tag="otc")
                    nc.sync.dma_start(ot_c[:], OT_dram[:, :, lsl])
                    for co in range(C // 512):
                        pps = ps_c.tile([128, 512], F32, tag="pps")
                        for kd in range(NPAIR):
                            nc.tensor.matmul(
                                pps[:],
                                ot_c[:, kd, :],
                                wpT_t[:, kd, co * 512:(co + 1) * 512],
                                start=(kd == 0),
                                stop=(kd == NPAIR - 1),
                            )
                        ob = ob_pool.tile([128, 512], F32, tag="ob")
                        nc.vector.tensor_copy(ob[:], pps[:])
                        nc.sync.dma_start(outp_d[lt, :, co * 512:(co + 1) * 512], ob[:])

    nc.compile()
    return nc


def _get_nc():
    if "nc" not in _built:
        _built["nc"] = _build(None)
    return _built["nc"]


def _rope_perm():
    """Within-head row permutation: quadrant-local [evens(16) | odds(16)]."""
    perm = np.empty(64, np.int64)
    for j in range(2):
        for i in range(32):
            perm[j * 32 + i] = 2 * (j * 16 + i) if i < 16 else 2 * (j * 16 + i - 16) + 1
    return perm


def _shard_inputs(x, cos, sin, w_qkv, w_proj):
    perm = _rope_perm()
    p = np.arange(128)
    quad, i = p // 32, p % 32
    pairidx = (quad % 2) * 16 + (i % 16)
    sign = np.where(i < 16, -1.0, 1.0).astype(np.float32)
    cos4 = np.ascontiguousarray(cos[:, pairidx].T)                  # [128, L]
    sin4 = np.ascontiguousarray((sin[:, pairidx] * sign[None, :]).T)

    in_maps = []
    for c in range(NCORES):
        b, hg = c // 2, c % 2
        xT = np.ascontiguousarray(
            x[b].T.reshape(C // 128, 128, L).transpose(1, 0, 2)
        )  # [p, kc, l]

        rows = np.empty((8, 128), np.int64)
        for ft in range(8):
            t = 0 if ft < 4 else 1
            pr = ft % 4
            for fi in range(128):
                head = hg * 8 + 2 * pr + (0 if fi < 64 else 1)
                rows[ft, fi] = t * C + head * D + perm[fi % 64]
        wq = w_qkv[rows.reshape(-1)].reshape(8, 128, C // 128, 128)  # [ft, f, kc, p]
        wqkT = np.ascontiguousarray(wq.transpose(0, 3, 2, 1))        # [ft, p, kc, f]

        wv = w_qkv[2 * C + hg * 512: 2 * C + hg * 512 + 512]         # [fv, c]
        wvT = np.ascontiguousarray(wv.T.reshape(C // 128, 128, 512))  # [kc, p, fv]

        wp = w_proj[:, hg * 512: hg * 512 + 512]                     # [co, d']
        wpT = np.ascontiguousarray(
            wp.T.reshape(4, 128, C).transpose(1, 0, 2)
        )  # [p, kd, co]

        in_maps.append(
            {"xT": xT, "wqkT": wqkT, "wvT": wvT, "wpT": wpT, "cos4": cos4, "sin4": sin4}
        )
    return in_maps


def kernel(x, cos, sin, w_qkv, w_proj, b_proj, _trace=False):
    from concourse.bass_utils import run_bass_kernel_spmd

    x = np.asarray(x, dtype=np.float32)
    cos = np.asarray(cos, dtype=np.float32)
    sin = np.asarray(sin, dtype=np.float32)
    w_qkv = np.asarray(w_qkv, dtype=np.float32)
    w_proj = np.asarray(w_proj, dtype=np.float32)
    b_proj = np.asarray(b_proj, dtype=np.float32)

    nc = _get_nc()
    in_maps = _shard_inputs(x, cos, sin, w_qkv, w_proj)
    res = run_bass_kernel_spmd(
        nc, in_maps, core_ids=list(range(NCORES)), trace=_trace
    )
    if _trace:
        print("exec_time_ns:", res.exec_time_ns)
        print("trace:", res.instructions_and_trace[1] if res.instructions_and_trace else None)

    out = np.empty((B, L, C), dtype=np.float32)
    for b in range(B):
        p0 = res.results[2 * b]["outp"].reshape(L, C)
        p1 = res.results[2 * b + 1]["outp"].reshape(L, C)
        out[b] = p0 + p1
    out += b_proj[None, None, :]
    return out

